# revision 51
# baseline (speedup 1.0000x reference)
"""Trainium2 Bass kernel for nn_ButterflyLayer2D (butterfly 2D CNN).

Strategy: pure data parallel over 8 NeuronCores (16 batch each).

Layouts (per core):
  - activations in SBUF as [128 = (w%2)*64 + c, (node, b, h, w//2)]; every
    2x2-stride-2 per-node conv is 4 bf16 matmuls with K=128=(y,ci):
    q = output w-parity goes to PE column-tile (0, q*64) so the two q
    streams run CONCURRENTLY on the two column halves of the PE array,
    x = input h-parity accumulates in PSUM.  PSUM rows are (q, c_out) ==
    exactly the next level's partition layout, so each eviction is ONE
    full-width relu(+bias) op with contiguous psum read and contiguous
    sbuf write (alternating ScalarE/VectorE).
  - the input 4x4-patch conv packs TWO horizontally adjacent patches per
    rhs column (K=32, block-diagonal lhsT, M=128=(w%2, c)) and row-tiles
    4 ways over (b%4); it is interleaved with level 1 per batch-pair so
    the eviction engines never idle.
  - level 6 (spatial 1x1) computes node pairs with M=(coA,coB);
    features land as [128=(sib,c), (pair, b)].
  - the final dense is row-tiled 2x (even nodes PE rows 0-63, odd 64-127)
    into parity-major psum tiles; output written parity-major and
    decoded on the host.
When all biases are zero (checked on host) psum tiles covering several
nodes are evicted in one op; otherwise per-node evicts apply the bias.
Weights are prefetched whole-level across both HWDGE rings; w3 streams
just-in-time in 8-node chunks.
"""

import numpy as np
from contextlib import ExitStack

import concourse.bass as bass
import concourse.tile as tile
from concourse import bacc, mybir
from concourse.bass_utils import run_bass_kernel_spmd

F32 = mybir.dt.float32
BF16 = mybir.dt.bfloat16
AF = mybir.ActivationFunctionType
ALU = mybir.AluOpType

B, IN, NLVL, KLVL, C = 128, 256, 6, 3, 64
NK, OU, OV = 8, 8, 8
NCORES = 8
BC = B // NCORES          # 16 per-core batch
PH = 1
BG = BC
TCOL = 1024               # psum tile columns (2 banks)
PBUFS = 4
LVL_NODES = [4, 16, 64, 64, 64, 64]
LVL_HIN = [64, 32, 16, 8, 4, 2]


# ----------------------------------------------------------------------------
# host-side pre-arrangement
# ----------------------------------------------------------------------------

def _prep_weights(inputs):
    """Weights/biases blobs shared by all cores."""
    import ml_dtypes
    out = {}
    # input filter: per (g=b%4, q=w%2) a [128, 64] lhsT that is zero except
    # rows g*32+q*16 .. +16 = fin[pix, c] — K=128 mms in the same
    # 128x64 column-tiled mode as the levels (no PE mode switches)
    fin = inputs["in_filter"][:, :, 0, :].reshape(16, C).astype(np.float32)
    finr = np.zeros((128, 8, C), np.float32)
    for g in range(4):
        for q in range(2):
            finr[g * 32 + q * 16 : g * 32 + (q + 1) * 16, g * 2 + q] = fin
    out["fin"] = finr.reshape(128, 8 * C).astype(ml_dtypes.bfloat16)
    # misc blob: [bin | b1 | b2 | b3 | b4 | b5 | b6]
    misc = [np.concatenate([inputs["in_bias"], inputs["in_bias"]]).reshape(128, 1)]
    for lvl in range(1, NLVL + 1):
        f = inputs[f"f{lvl}"].astype(np.float32)  # [n,n,2,2,C,C] (x,y,ci,co)
        n = f.shape[0]
        # per node lhsT [(y,ci)=128, (x,co)=128]
        w = f.transpose(0, 1, 3, 4, 2, 5).reshape(n * n, 2 * C, 2 * C)
        out[f"w{lvl}"] = np.ascontiguousarray(w.transpose(1, 0, 2)).reshape(
            128, n * n * 128
        ).astype(ml_dtypes.bfloat16)
        b = inputs[f"b{lvl}"].astype(np.float32).reshape(n * n, C)
        if lvl < NLVL:
            misc.append(np.concatenate([b, b], axis=1).T)  # [(q,c)=128, nodes]
        else:
            misc.append(b.reshape(n * n // 2, 2 * C).T)    # [(cA,cB), pairs]
    out["misc"] = np.ascontiguousarray(
        np.concatenate(misc, axis=1)
    ).astype(np.float32)
    # dense: [128, pairs*128]: rows 0-63 even-node [c,(r,ouov)], 64-127 odd
    wd = inputs["Wd"].astype(np.float32).reshape(NK * NK, 2, C, OU * OV)
    wd = wd.transpose(2, 0, 1, 3).reshape(C, NK * NK, 2 * OU * OV)
    wd2 = np.zeros((128, NK * NK // 2 * 128), np.float32)
    for k in range(NK * NK // 2):
        wd2[0:64, k * 128 : (k + 1) * 128] = wd[:, 2 * k]
        wd2[64:128, k * 128 : (k + 1) * 128] = wd[:, 2 * k + 1]
    out["wd"] = wd2.astype(ml_dtypes.bfloat16)
    return out


def _prep_input(in_data_core):
    """Per-core input blob [128 = (b%4)*32 + (j%8//4)*16 + (i%4)*4 + (j%4),
    (b//4, x=i//4, y2=j//8)]."""
    import ml_dtypes
    ind = in_data_core[:, :, :, 0]  # [16, 256, 256]
    a = ind.reshape(4, 4, 64, 4, 32, 2, 4)  # [half, g, x, p, y2, yp, q]
    a = a.transpose(1, 5, 3, 6, 0, 2, 4)    # [g, yp, p, q, half, x, y2]
    return np.ascontiguousarray(a).reshape(128, 4 * 64 * 32).astype(
        ml_dtypes.bfloat16
    )


def _decode_output(t2_core):
    """t2 [128=(r,ou,ov), (s, k, b)] with node = 2k+s -> [16, 64, 64, 2]."""
    t = t2_core.reshape(2, OU, OV, 2, 8, 4, BG)  # r,ou,ov,s,u,v2,b
    t = t.transpose(6, 4, 1, 5, 3, 2, 0)         # b,u,ou,v2,s,ov,r
    return np.ascontiguousarray(t).reshape(BC, NK * OU, NK * OV, 2)


# ----------------------------------------------------------------------------
# device kernel
# ----------------------------------------------------------------------------

def _build_kernel(zero_bias=True):
    nc = bacc.Bacc(None, target_bir_lowering=False)
    p = {}
    p["a0"] = nc.declare_dram_parameter("a0", [128, 4 * 64 * 32], BF16, isOutput=False)
    p["fin"] = nc.declare_dram_parameter("fin", [128, 512], BF16, isOutput=False)
    # misc f32 blob: [bin(1) | b1(4) | b2(16) | b3..b5(64 each) | b6(32)]
    p["misc"] = nc.declare_dram_parameter("misc", [128, 245], F32, isOutput=False)
    for lvl in range(1, NLVL + 1):
        n2 = LVL_NODES[lvl - 1]
        p[f"w{lvl}"] = nc.declare_dram_parameter(f"w{lvl}", [128, n2 * 128], BF16, isOutput=False)
    p["wd"] = nc.declare_dram_parameter("wd", [128, NK * NK // 2 * 128], BF16, isOutput=False)
    t2 = nc.declare_dram_parameter("t2", [128, NK * NK * BG], BF16, isOutput=True)

    evict_ctr = [0]

    def evict(out_ap, psum_ap, bias_ap=None):
        """relu(psum [+ bias]) -> sbuf, alternating engines."""
        evict_ctr[0] += 1
        if evict_ctr[0] % 13 % 2 == 0:
            if bias_ap is None:
                nc.scalar.activation(out_ap, psum_ap, AF.Relu)
            else:
                nc.scalar.activation(out_ap, psum_ap, AF.Relu, bias=bias_ap)
        elif bias_ap is None:
            nc.vector.tensor_scalar_max(out_ap, psum_ap, 0.0)
        else:
            nc.vector.tensor_scalar(out_ap, psum_ap, bias_ap, 0.0,
                                    op0=ALU.add, op1=ALU.max)

    with tile.TileContext(nc) as tc, ExitStack() as ctx:
        const = ctx.enter_context(tc.tile_pool(name="const", bufs=1))
        wbig = ctx.enter_context(tc.tile_pool(name="wbig", bufs=1))
        wpool = ctx.enter_context(tc.tile_pool(name="wts", bufs=4))
        apool = ctx.enter_context(tc.tile_pool(name="acts", bufs=1))
        inpool = ctx.enter_context(tc.tile_pool(name="inp", bufs=1))
        fpool = ctx.enter_context(tc.tile_pool(name="feat", bufs=1))
        ppool = ctx.enter_context(tc.tile_pool(name="ps", bufs=PBUFS, space="PSUM"))

        # ---------------- DMA prologue ----------------
        # scalar ring (fast q10): a0 halves 0,1 then big weights w5,w6,wd,w4
        # sync ring: consts, w1, w2; later a0 halves 2,3 + w3 chunks in-loop
        a0s = inpool.tile([128, 2 * 64 * 32], BF16, tag="a0s", name="a0s")
        # scalar ring (fast) carries the critical stream in dependency
        # order: input half 0, w1, input half 1, w2; big late weights are
        # issued from inside the pipeline loop so they queue behind the
        # staging transfers, not ahead of them.
        nc.scalar.dma_start(a0s[:, 0:1024], p["a0"][:, 0:1024])
        fin_t = const.tile([128, 512], BF16)
        nc.sync.dma_start(fin_t[:], p["fin"][:])
        misc_t = const.tile([128, 245], F32, tag="misc", name="misc")
        nc.sync.dma_start(misc_t[:], p["misc"][:])
        bin_t = misc_t[:, 0:1]
        bias_t = {}
        boff = 1
        for lvl in range(1, NLVL + 1):
            bcols = LVL_NODES[lvl - 1] if lvl < NLVL else LVL_NODES[lvl - 1] // 2
            bias_t[lvl] = misc_t[:, boff : boff + bcols]
            boff += bcols
        nc.scalar.dma_start(a0s[:, 1024:2048], p["a0"][:, 1024:2048])
        w_t = {}
        w_t[1] = wbig.tile([128, 4 * 128], BF16, tag="w1", name="w1")
        nc.scalar.dma_start(w_t[1][:], p["w1"][:])
        nc.scalar.dma_start(a0s[:, 2048:4096], p["a0"][:, 2048:4096])
        w_t[2] = wbig.tile([128, 16 * 128], BF16, tag="w2", name="w2")
        nc.scalar.dma_start(w_t[2][:], p["w2"][:])
        # big late weights ride the otherwise-idle sync ring in the background
        w_t[6] = wbig.tile([128, 64 * 128], BF16, tag="w6", name="w6")
        nc.sync.dma_start(w_t[6][:], p["w6"][:])
        wd_t = wbig.tile([128, NK * NK // 2 * 128], BF16, tag="wd", name="wd")
        nc.sync.dma_start(wd_t[:], p["wd"][:])

        a0v = a0s[:].rearrange("p (h x y) -> p h x y", h=2, x=64)

        # ------- input conv + level 1 + level 2, software-pipelined -------
        # X is a 4-slice ring over b%4 (L1 consumes each b right away):
        # [128=(w%2,c), (b%4, h=64, w2=32)]; L1 out: [128, (n=4, b, 32, 16)]
        # L2 out: [128, (n=16, b, 16, 8)]
        X = apool.tile([128, 4 * 64 * 32], BF16, tag="sx", name="x0")
        X2d = X[:]
        Xv = X[:].rearrange("p (b h w) -> p b h w", b=4, h=64)
        a1 = apool.tile([128, 4 * BG * 32 * 16], BF16, tag="s1", name="a1")
        a12d = a1[:]
        a1v = a1[:].rearrange("p (n b h w) -> p n b h w", n=4, b=BG, h=32)
        a2 = apool.tile([128, 16 * BG * 16 * 8], BF16, tag="s2", name="a2")
        a22d = a2[:]

        def input_pair(pr):
            for bl in (2 * pr, 2 * pr + 1):
                g, hh = bl % 4, (bl // 4) % 2
                for xh in range(2):
                    pt = ppool.tile([128, TCOL], F32, tag="ps",
                                    padded_shape=[128, TCOL],
                                    name=f"pin_{bl}_{xh}")
                    for sub in range(2):
                        rhs = a0v[:, hh,
                                  xh * 32 + sub * 16 : xh * 32 + (sub + 1) * 16, :]
                        for q in (0, 1):
                            nc.tensor.matmul(
                                pt[q * 64 : (q + 1) * 64,
                                   sub * 512 : (sub + 1) * 512],
                                fin_t[:, (g * 2 + q) * 64 : (g * 2 + q + 1) * 64],
                                rhs,
                                start=True, stop=True,
                                tile_position=(0, q * 64),
                            )
                    evict(
                        X2d[:, ((bl % 4) * 64 + xh * 32) * 32 :
                            ((bl % 4) * 64 + (xh + 1) * 32) * 32],
                        pt[:], bin_t,
                    )

        def l1_tiles(pr):
            # 4 nodes for batch pair pr (parent is the whole X)
            bs = 2 * pr
            for node in range(4):
                pt = ppool.tile([128, TCOL], F32, tag="ps",
                                padded_shape=[128, TCOL],
                                name=f"p1_{node}_{pr}")
                for x in (0, 1):
                    for q in (0, 1):
                        for sub in (0, 1):
                            rhs = Xv[:, (bs + sub) % 4, x::2, q::2]
                            nc.tensor.matmul(
                                pt[q * 64 : (q + 1) * 64,
                                   sub * 512 : (sub + 1) * 512],
                                w_t[1][:, node * 128 + x * 64 :
                                       node * 128 + (x + 1) * 64],
                                rhs,
                                start=(x == 0), stop=(x == 1),
                                skip_group_check=True,
                                tile_position=(0, q * 64),
                            )
                evict(
                    a12d[:, (node * BG + bs) * 512 : (node * BG + bs + 2) * 512],
                    pt[:],
                    bias_t[1][:, node : node + 1],
                )

        def l2_tile(node, bs):
            # one [128,1024] tile: 8 batches of one node; parent in a1
            pn = (node // 4 // 2) * 2 + (node % 4) // 2
            pt = ppool.tile([128, TCOL], F32, tag="ps",
                            padded_shape=[128, TCOL],
                            name=f"p2_{node}_{bs}")
            for x in (0, 1):
                for q in (0, 1):
                    for sub in (0, 1):
                        b0 = bs + sub * 4
                        rhs = a1v[:, pn, b0 : b0 + 4, x::2, q::2]
                        nc.tensor.matmul(
                            pt[q * 64 : (q + 1) * 64,
                               sub * 512 : (sub + 1) * 512],
                            w_t[2][:, node * 128 + x * 64 :
                                   node * 128 + (x + 1) * 64],
                            rhs,
                            start=(x == 0), stop=(x == 1),
                            skip_group_check=True,
                            tile_position=(0, q * 64),
                        )
            evict(
                a22d[:, (node * BG + bs) * 128 : (node * BG + bs + 8) * 128],
                pt[:],
                bias_t[2][:, node : node + 1],
            )

        for pr in range(8):
            # stage the next a0 slot as soon as the previous pair freed it
            if pr == 2:
                nc.scalar.dma_start(a0s[:, 0:2048], p["a0"][:, 4096:6144])
            elif pr == 4:
                nc.scalar.dma_start(a0s[:, 2048:4096], p["a0"][:, 6144:8192])

            input_pair(pr)
            l1_tiles(pr)
            if pr >= 4:
                # L1 b0-7 is complete; start L2 (bs=0) tiles
                for node in range(4 * (pr - 4), 4 * (pr - 3)):
                    l2_tile(node, 0)
        for node in range(16):
            l2_tile(node, 8)

        # ---------------- levels 3..5 (q-scheme) ----------------
        cur, cur_nodes = a2, 16
        tags = ["s1", "s2", "s1"]
        for lvl in range(3, 6):
            n2 = LVL_NODES[lvl - 1]
            grid = int(np.sqrt(n2))
            Hin = LVL_HIN[lvl - 1]
            Ho, W2o = Hin // 2, Hin // 4
            cpn = BG * Ho * W2o          # psum cols per node
            pgrid = int(np.sqrt(cur_nodes))
            nxt = apool.tile([128, n2 * cpn], BF16, tag=tags[lvl - 3],
                             name=f"a{lvl}")
            nxt2d = nxt[:]
            curv = cur[:].rearrange("p (n b h w) -> p n b h w",
                                    n=cur_nodes, b=BG, h=Hin)

            def parent(node):
                if lvl > KLVL:
                    return node
                u, v = node // grid, node % grid
                return (u // 2) * pgrid + (v // 2)

            if cpn >= TCOL:
                # large nodes (L2: 2 tiles/node split along b)
                tpn = cpn // TCOL
                bper = TCOL // (Ho * W2o)
                bh = bper // 2
                for node in range(n2):
                    pn = parent(node)
                    for t in range(tpn):
                        bs = t * bper
                        pt = ppool.tile([128, TCOL], F32, tag="ps",
                                        padded_shape=[128, TCOL],
                                        name=f"p{lvl}_{node}_{t}")
                        for x in (0, 1):
                            for q in (0, 1):
                                for sub in (0, 1):
                                    b0 = bs + sub * bh
                                    rhs = curv[:, pn, b0 : b0 + bh, x::2, q::2]
                                    nc.tensor.matmul(
                                        pt[q * 64 : (q + 1) * 64,
                                           sub * 512 : (sub + 1) * 512],
                                        w_t[lvl][:, node * 128 + x * 64 :
                                                 node * 128 + (x + 1) * 64],
                                        rhs,
                                        start=(x == 0), stop=(x == 1),
                                        skip_group_check=True,
                                        tile_position=(0, q * 64),
                                    )
                        evict(
                            nxt2d[:, (node * BG + bs) * Ho * W2o :
                                  (node * BG + bs + bper) * Ho * W2o],
                            pt[:],
                            bias_t[lvl][:, node : node + 1],
                        )
            else:
                # small nodes: multiple nodes per psum tile
                npt = TCOL // cpn        # L3: 2, L4: 8, L5: 32
                wch = None
                for n0 in range(0, n2, npt):
                    pt = ppool.tile([128, npt * cpn], F32, tag="ps",
                                    padded_shape=[128, TCOL],
                                    name=f"p{lvl}_{n0}")
                    for ln in range(npt):
                        node = n0 + ln
                        pn = parent(node)
                        if lvl in (3, 4, 5):
                            # w3/w4/w5 stream just-in-time in 16-node chunks
                            # on the fast (scalar) ring
                            if node % 16 == 0:
                                wch = wpool.tile([128, 2048], BF16, tag="wch",
                                                 name=f"w{lvl}c_{node}")
                                nc.scalar.dma_start(
                                    wch[:],
                                    p[f"w{lvl}"][:, node * 128 : (node + 16) * 128],
                                )
                            wof = (node % 16) * 128
                            wsl = wch
                        else:
                            wof = node * 128
                            wsl = w_t[lvl]
                        for x in (0, 1):
                            for q in (0, 1):
                                rhs = curv[:, pn, :, x::2, q::2]
                                nc.tensor.matmul(
                                    pt[q * 64 : (q + 1) * 64,
                                       ln * cpn : (ln + 1) * cpn],
                                    wsl[:, wof + x * 64 : wof + (x + 1) * 64],
                                    rhs,
                                    start=(x == 0), stop=(x == 1),
                                    skip_group_check=True,
                                    tile_position=(0, q * 64),
                                )
                    if zero_bias:
                        evict(nxt2d[:, n0 * cpn : (n0 + npt) * cpn],
                              pt[:, : npt * cpn])
                    else:
                        for ln in range(npt):
                            node = n0 + ln
                            evict(
                                nxt2d[:, node * cpn : (node + 1) * cpn],
                                pt[:, ln * cpn : (ln + 1) * cpn],
                                bias_t[lvl][:, node : node + 1],
                            )
            cur, cur_nodes = nxt, n2

        # ---------------- level 6 (node pairs, 1x1 out) ----------------
        # feats F2 [128=(sib,c), (pair, b)]
        cur5v = cur[:].rearrange("p (n b h w) -> p n b h w", n=64, b=BG, h=2)
        F2 = fpool.tile([128, 32 * BG], BF16, tag="feats", name="feats")
        F2v = F2[:].rearrange("p (r b) -> p r b", r=32)
        for p0 in range(0, 32, 16):
            pt6 = ppool.tile([128, 16 * BG], F32, tag="ps",
                             padded_shape=[128, TCOL], name=f"p6_{p0}")
            for pr in range(p0, p0 + 16):
                lp = pr - p0
                for x in (0, 1):
                    for half in (0, 1):
                        node = 2 * pr + half
                        rhs = cur5v[:, node, :, x, 0]
                        nc.tensor.matmul(
                            pt6[half * 64 : (half + 1) * 64,
                                lp * BG : (lp + 1) * BG],
                            w_t[6][:, node * 128 + x * 64 :
                                   node * 128 + (x + 1) * 64],
                            rhs,
                            start=(x == 0), stop=(x == 1),
                            skip_group_check=True,
                            tile_position=(0, half * 64),
                        )
            if zero_bias:
                evict(F2[:][:, p0 * BG : (p0 + 16) * BG], pt6[:])
            else:
                for pr in range(p0, p0 + 16):
                    lp = pr - p0
                    evict(F2v[:, pr, :], pt6[:, lp * BG : (lp + 1) * BG],
                          bias_t[6][:, pr : pr + 1])

        # ---------------- dense (row-tiled 2x, parity-major out) ----------
        t2s = fpool.tile([128, NK * NK * BG], BF16, tag="t2s", name="t2s")
        for half in range(2):
            ptd = [
                ppool.tile([128, 256], F32, tag="ps",
                           padded_shape=[128, TCOL], name=f"pd_{s}_{half}")
                for s in range(2)
            ]
            for k in range(half * 16, (half + 1) * 16):
                for s in range(2):
                    rhs = F2v[s * 64 : (s + 1) * 64, k, :]
                    nc.tensor.matmul(
                        ptd[s][:, (k % 16) * BG : (k % 16 + 1) * BG],
                        wd_t[s * 64 : (s + 1) * 64, k * 128 : (k + 1) * 128],
                        rhs,
                        start=True, stop=True,
                        tile_position=(s * 64, 0),
                    )
            for s in range(2):
                dst = t2s[:, s * 512 + half * 256 : s * 512 + (half + 1) * 256]
                if s == 0:
                    nc.scalar.copy(dst, ptd[s][:])
                else:
                    nc.vector.tensor_copy(dst, ptd[s][:])
                nc.scalar.dma_start(
                    t2[:, s * 512 + half * 256 : s * 512 + (half + 1) * 256], dst
                )
    nc.compile()
    return nc


# ----------------------------------------------------------------------------
# entry point
# ----------------------------------------------------------------------------

def kernel(**inputs):
    inputs = {k: np.asarray(v) for k, v in inputs.items()}
    zb = not np.any(inputs["in_bias"]) and all(
        not np.any(inputs[f"b{l}"]) for l in range(1, NLVL + 1)
    )
    wblobs = _prep_weights(inputs)
    nc = _build_kernel(zero_bias=zb)
    in_maps = []
    for c in range(NCORES):
        m = dict(wblobs)
        m["a0"] = _prep_input(inputs["in_data"][c * BC : (c + 1) * BC])
        in_maps.append(m)
    res = run_bass_kernel_spmd(nc, in_maps, list(range(NCORES)))
    outs = [_decode_output(res.results[c]["t2"]) for c in range(NCORES)]
    return np.concatenate(outs, axis=0).astype(np.float32)


if __name__ == "__main__":
    import reference as ref

    inputs = {k: np.asarray(v) for k, v in ref.setup_inputs().items()}
    expected = np.asarray(ref.reference(**inputs))
    actual = kernel(**inputs)
    err = np.abs(actual - expected).max()
    rel = err / np.abs(expected).max()
    print("absmax:", err, "rel:", rel)


# revision 53
# speedup vs baseline: 1.0232x; 1.0232x over previous
"""Trainium2 Bass kernel for nn_ButterflyLayer2D (butterfly 2D CNN).

Strategy: pure data parallel over 8 NeuronCores (16 batch each).

Layouts (per core):
  - activations in SBUF as [128 = (w%2)*64 + c, (node, b, h, w//2)]; every
    2x2-stride-2 per-node conv is 4 bf16 matmuls with K=128=(y,ci):
    q = output w-parity goes to PE column-tile (0, q*64) so the two q
    streams run CONCURRENTLY on the two column halves of the PE array,
    x = input h-parity accumulates in PSUM.  PSUM rows are (q, c_out) ==
    exactly the next level's partition layout, so each eviction is ONE
    full-width relu(+bias) op with contiguous psum read and contiguous
    sbuf write (alternating ScalarE/VectorE).
  - the input 4x4-patch conv packs TWO horizontally adjacent patches per
    rhs column (K=32, block-diagonal lhsT, M=128=(w%2, c)) and row-tiles
    4 ways over (b%4); it is interleaved with level 1 per batch-pair so
    the eviction engines never idle.
  - level 6 (spatial 1x1) computes node pairs with M=(coA,coB);
    features land as [128=(sib,c), (pair, b)].
  - the final dense is row-tiled 2x (even nodes PE rows 0-63, odd 64-127)
    into parity-major psum tiles; output written parity-major and
    decoded on the host.
When all biases are zero (checked on host) psum tiles covering several
nodes are evicted in one op; otherwise per-node evicts apply the bias.
Weights are prefetched whole-level across both HWDGE rings; w3 streams
just-in-time in 8-node chunks.
"""

import numpy as np
from contextlib import ExitStack

import concourse.bass as bass
import concourse.tile as tile
from concourse import bacc, mybir
from concourse.bass_utils import run_bass_kernel_spmd

F32 = mybir.dt.float32
BF16 = mybir.dt.bfloat16
AF = mybir.ActivationFunctionType
ALU = mybir.AluOpType

B, IN, NLVL, KLVL, C = 128, 256, 6, 3, 64
NK, OU, OV = 8, 8, 8
NCORES = 8
BC = B // NCORES          # 16 per-core batch
PH = 1
BG = BC
TCOL = 1024               # psum tile columns (2 banks)
PBUFS = 4
LVL_NODES = [4, 16, 64, 64, 64, 64]
LVL_HIN = [64, 32, 16, 8, 4, 2]


# ----------------------------------------------------------------------------
# host-side pre-arrangement
# ----------------------------------------------------------------------------

def _prep_weights(inputs):
    """Weights/biases blobs shared by all cores."""
    import ml_dtypes
    out = {}
    # input filter: per (g=b%4, q=w%2) a [128, 64] lhsT that is zero except
    # rows g*32+q*16 .. +16 = fin[pix, c] — K=128 mms in the same
    # 128x64 column-tiled mode as the levels (no PE mode switches)
    fin = inputs["in_filter"][:, :, 0, :].reshape(16, C).astype(np.float32)
    finr = np.zeros((128, 8, C), np.float32)
    for g in range(4):
        for q in range(2):
            finr[g * 32 + q * 16 : g * 32 + (q + 1) * 16, g * 2 + q] = fin
    out["fin"] = finr.reshape(128, 8 * C).astype(ml_dtypes.bfloat16)
    # misc blob: [bin | b1 | b2 | b3 | b4 | b5 | b6]
    misc = [np.concatenate([inputs["in_bias"], inputs["in_bias"]]).reshape(128, 1)]
    for lvl in range(1, NLVL + 1):
        f = inputs[f"f{lvl}"].astype(np.float32)  # [n,n,2,2,C,C] (x,y,ci,co)
        n = f.shape[0]
        # per node lhsT [(y,ci)=128, (x,co)=128]
        w = f.transpose(0, 1, 3, 4, 2, 5).reshape(n * n, 2 * C, 2 * C)
        out[f"w{lvl}"] = np.ascontiguousarray(w.transpose(1, 0, 2)).reshape(
            128, n * n * 128
        ).astype(ml_dtypes.bfloat16)
        b = inputs[f"b{lvl}"].astype(np.float32).reshape(n * n, C)
        if lvl < NLVL:
            misc.append(np.concatenate([b, b], axis=1).T)  # [(q,c)=128, nodes]
        else:
            misc.append(b.reshape(n * n // 2, 2 * C).T)    # [(cA,cB), pairs]
    out["misc"] = np.ascontiguousarray(
        np.concatenate(misc, axis=1)
    ).astype(np.float32)
    # dense: [128, pairs*128]: rows 0-63 even-node [c,(r,ouov)], 64-127 odd
    wd = inputs["Wd"].astype(np.float32).reshape(NK * NK, 2, C, OU * OV)
    wd = wd.transpose(2, 0, 1, 3).reshape(C, NK * NK, 2 * OU * OV)
    wd2 = np.zeros((128, NK * NK // 2 * 128), np.float32)
    for k in range(NK * NK // 2):
        wd2[0:64, k * 128 : (k + 1) * 128] = wd[:, 2 * k]
        wd2[64:128, k * 128 : (k + 1) * 128] = wd[:, 2 * k + 1]
    out["wd"] = wd2.astype(ml_dtypes.bfloat16)
    return out


def _prep_input(in_data_core):
    """Per-core input blob [128 = (b%4)*32 + (j%8//4)*16 + (i%4)*4 + (j%4),
    (b//4, x=i//4, y2=j//8)]."""
    import ml_dtypes
    ind = in_data_core[:, :, :, 0]  # [16, 256, 256]
    a = ind.reshape(4, 4, 64, 4, 32, 2, 4)  # [half, g, x, p, y2, yp, q]
    a = a.transpose(1, 5, 3, 6, 0, 2, 4)    # [g, yp, p, q, half, x, y2]
    return np.ascontiguousarray(a).reshape(128, 4 * 64 * 32).astype(
        ml_dtypes.bfloat16
    )


def _decode_output(t2_core):
    """t2 [128=(r,ou,ov), (s, k, b)] with node = 2k+s -> [16, 64, 64, 2]."""
    t = t2_core.reshape(2, OU, OV, 2, 8, 4, BG)  # r,ou,ov,s,u,v2,b
    t = t.transpose(6, 4, 1, 5, 3, 2, 0)         # b,u,ou,v2,s,ov,r
    return np.ascontiguousarray(t).reshape(BC, NK * OU, NK * OV, 2)


# ----------------------------------------------------------------------------
# device kernel
# ----------------------------------------------------------------------------

def _build_kernel(zero_bias=True):
    nc = bacc.Bacc(None, target_bir_lowering=False)
    p = {}
    p["a0"] = nc.declare_dram_parameter("a0", [128, 4 * 64 * 32], BF16, isOutput=False)
    p["fin"] = nc.declare_dram_parameter("fin", [128, 512], BF16, isOutput=False)
    # misc f32 blob: [bin(1) | b1(4) | b2(16) | b3..b5(64 each) | b6(32)]
    p["misc"] = nc.declare_dram_parameter("misc", [128, 245], F32, isOutput=False)
    for lvl in range(1, NLVL + 1):
        n2 = LVL_NODES[lvl - 1]
        p[f"w{lvl}"] = nc.declare_dram_parameter(f"w{lvl}", [128, n2 * 128], BF16, isOutput=False)
    p["wd"] = nc.declare_dram_parameter("wd", [128, NK * NK // 2 * 128], BF16, isOutput=False)
    t2 = nc.declare_dram_parameter("t2", [128, NK * NK * BG], BF16, isOutput=True)

    evict_ctr = [0]

    def evict(out_ap, psum_ap, bias_ap=None):
        """relu(psum [+ bias]) -> sbuf, alternating engines."""
        evict_ctr[0] += 1
        if evict_ctr[0] % 13 % 2 == 0:
            if bias_ap is None:
                nc.scalar.activation(out_ap, psum_ap, AF.Relu)
            else:
                nc.scalar.activation(out_ap, psum_ap, AF.Relu, bias=bias_ap)
        elif bias_ap is None:
            nc.vector.tensor_scalar_max(out_ap, psum_ap, 0.0)
        else:
            nc.vector.tensor_scalar(out_ap, psum_ap, bias_ap, 0.0,
                                    op0=ALU.add, op1=ALU.max)

    with tile.TileContext(nc) as tc, ExitStack() as ctx:
        const = ctx.enter_context(tc.tile_pool(name="const", bufs=1))
        wbig = ctx.enter_context(tc.tile_pool(name="wbig", bufs=1))
        wpool = ctx.enter_context(tc.tile_pool(name="wts", bufs=4))
        apool = ctx.enter_context(tc.tile_pool(name="acts", bufs=1))
        inpool = ctx.enter_context(tc.tile_pool(name="inp", bufs=1))
        fpool = ctx.enter_context(tc.tile_pool(name="feat", bufs=1))
        ppool = ctx.enter_context(tc.tile_pool(name="ps", bufs=PBUFS, space="PSUM"))

        # ---------------- DMA prologue ----------------
        # scalar ring (fast q10): a0 halves 0,1 then big weights w5,w6,wd,w4
        # sync ring: consts, w1, w2; later a0 halves 2,3 + w3 chunks in-loop
        a0s = inpool.tile([128, 2 * 64 * 32], BF16, tag="a0s", name="a0s")
        # scalar ring (fast) carries the critical stream in dependency
        # order: input half 0, w1, input half 1, w2; big late weights are
        # issued from inside the pipeline loop so they queue behind the
        # staging transfers, not ahead of them.
        nc.scalar.dma_start(a0s[:, 0:1024], p["a0"][:, 0:1024])
        fin_t = const.tile([128, 512], BF16)
        nc.sync.dma_start(fin_t[:], p["fin"][:])
        misc_t = const.tile([128, 245], F32, tag="misc", name="misc")
        nc.sync.dma_start(misc_t[:], p["misc"][:])
        bin_t = misc_t[:, 0:1]
        bias_t = {}
        boff = 1
        for lvl in range(1, NLVL + 1):
            bcols = LVL_NODES[lvl - 1] if lvl < NLVL else LVL_NODES[lvl - 1] // 2
            bias_t[lvl] = misc_t[:, boff : boff + bcols]
            boff += bcols
        nc.scalar.dma_start(a0s[:, 1024:2048], p["a0"][:, 1024:2048])
        w_t = {}
        w_t[1] = wbig.tile([128, 4 * 128], BF16, tag="w1", name="w1")
        nc.scalar.dma_start(w_t[1][:], p["w1"][:])
        nc.scalar.dma_start(a0s[:, 2048:4096], p["a0"][:, 2048:4096])
        w_t[2] = wbig.tile([128, 16 * 128], BF16, tag="w2", name="w2")
        nc.scalar.dma_start(w_t[2][:], p["w2"][:])
        w_t[6] = wbig.tile([128, 64 * 128], BF16, tag="w6", name="w6")
        wd_t = wbig.tile([128, NK * NK // 2 * 128], BF16, tag="wd", name="wd")

        a0v = a0s[:].rearrange("p (h x y) -> p h x y", h=2, x=64)

        # ------- input conv + level 1 + level 2, software-pipelined -------
        # X is a 4-slice ring over b%4 (L1 consumes each b right away):
        # [128=(w%2,c), (b%4, h=64, w2=32)]; L1 out: [128, (n=4, b, 32, 16)]
        # L2 out: [128, (n=16, b, 16, 8)]
        X = apool.tile([128, 4 * 64 * 32], BF16, tag="sx", name="x0")
        X2d = X[:]
        Xv = X[:].rearrange("p (b h w) -> p b h w", b=4, h=64)
        a1 = apool.tile([128, 4 * BG * 32 * 16], BF16, tag="s1", name="a1")
        a12d = a1[:]
        a1v = a1[:].rearrange("p (n b h w) -> p n b h w", n=4, b=BG, h=32)
        a2 = apool.tile([128, 16 * BG * 16 * 8], BF16, tag="s2", name="a2")
        a22d = a2[:]

        def input_pair(pr):
            for bl in (2 * pr, 2 * pr + 1):
                g, hh = bl % 4, (bl // 4) % 2
                for xh in range(2):
                    pt = ppool.tile([128, TCOL], F32, tag="ps",
                                    padded_shape=[128, TCOL],
                                    name=f"pin_{bl}_{xh}")
                    for sub in range(2):
                        rhs = a0v[:, hh,
                                  xh * 32 + sub * 16 : xh * 32 + (sub + 1) * 16, :]
                        for q in (0, 1):
                            nc.tensor.matmul(
                                pt[q * 64 : (q + 1) * 64,
                                   sub * 512 : (sub + 1) * 512],
                                fin_t[:, (g * 2 + q) * 64 : (g * 2 + q + 1) * 64],
                                rhs,
                                start=True, stop=True,
                                tile_position=(0, q * 64),
                            )
                    evict(
                        X2d[:, ((bl % 4) * 64 + xh * 32) * 32 :
                            ((bl % 4) * 64 + (xh + 1) * 32) * 32],
                        pt[:], bin_t,
                    )

        def l1_tiles(pr):
            # 4 nodes for batch pair pr (parent is the whole X)
            bs = 2 * pr
            for node in range(4):
                pt = ppool.tile([128, TCOL], F32, tag="ps",
                                padded_shape=[128, TCOL],
                                name=f"p1_{node}_{pr}")
                for x in (0, 1):
                    for q in (0, 1):
                        for sub in (0, 1):
                            rhs = Xv[:, (bs + sub) % 4, x::2, q::2]
                            nc.tensor.matmul(
                                pt[q * 64 : (q + 1) * 64,
                                   sub * 512 : (sub + 1) * 512],
                                w_t[1][:, node * 128 + x * 64 :
                                       node * 128 + (x + 1) * 64],
                                rhs,
                                start=(x == 0), stop=(x == 1),
                                skip_group_check=True,
                                tile_position=(0, q * 64),
                            )
                evict(
                    a12d[:, (node * BG + bs) * 512 : (node * BG + bs + 2) * 512],
                    pt[:],
                    bias_t[1][:, node : node + 1],
                )

        def l2_tile(node, bs):
            # one [128,1024] tile: 8 batches of one node; parent in a1
            pn = (node // 4 // 2) * 2 + (node % 4) // 2
            pt = ppool.tile([128, TCOL], F32, tag="ps",
                            padded_shape=[128, TCOL],
                            name=f"p2_{node}_{bs}")
            for x in (0, 1):
                for q in (0, 1):
                    for sub in (0, 1):
                        b0 = bs + sub * 4
                        rhs = a1v[:, pn, b0 : b0 + 4, x::2, q::2]
                        nc.tensor.matmul(
                            pt[q * 64 : (q + 1) * 64,
                               sub * 512 : (sub + 1) * 512],
                            w_t[2][:, node * 128 + x * 64 :
                                   node * 128 + (x + 1) * 64],
                            rhs,
                            start=(x == 0), stop=(x == 1),
                            skip_group_check=True,
                            tile_position=(0, q * 64),
                        )
            evict(
                a22d[:, (node * BG + bs) * 128 : (node * BG + bs + 8) * 128],
                pt[:],
                bias_t[2][:, node : node + 1],
            )

        for pr in range(8):
            # stage the next a0 slot as soon as the previous pair freed it
            if pr == 2:
                nc.scalar.dma_start(a0s[:, 0:2048], p["a0"][:, 4096:6144])
            elif pr == 4:
                nc.scalar.dma_start(a0s[:, 2048:4096], p["a0"][:, 6144:8192])
            elif pr == 5:
                # big late weights queue on the fast ring behind the staging
                nc.scalar.dma_start(w_t[6][:], p["w6"][:])
                nc.scalar.dma_start(wd_t[:], p["wd"][:])

            input_pair(pr)
            l1_tiles(pr)
            if pr >= 4:
                # L1 b0-7 is complete; start L2 (bs=0) tiles
                for node in range(4 * (pr - 4), 4 * (pr - 3)):
                    l2_tile(node, 0)
        for node in range(16):
            l2_tile(node, 8)

        # ---------------- levels 3..5 (q-scheme) ----------------
        cur, cur_nodes = a2, 16
        tags = ["s1", "s2", "s1"]
        for lvl in range(3, 6):
            n2 = LVL_NODES[lvl - 1]
            grid = int(np.sqrt(n2))
            Hin = LVL_HIN[lvl - 1]
            Ho, W2o = Hin // 2, Hin // 4
            cpn = BG * Ho * W2o          # psum cols per node
            pgrid = int(np.sqrt(cur_nodes))
            nxt = apool.tile([128, n2 * cpn], BF16, tag=tags[lvl - 3],
                             name=f"a{lvl}")
            nxt2d = nxt[:]
            curv = cur[:].rearrange("p (n b h w) -> p n b h w",
                                    n=cur_nodes, b=BG, h=Hin)

            def parent(node):
                if lvl > KLVL:
                    return node
                u, v = node // grid, node % grid
                return (u // 2) * pgrid + (v // 2)

            if cpn >= TCOL:
                # large nodes (L2: 2 tiles/node split along b)
                tpn = cpn // TCOL
                bper = TCOL // (Ho * W2o)
                bh = bper // 2
                for node in range(n2):
                    pn = parent(node)
                    for t in range(tpn):
                        bs = t * bper
                        pt = ppool.tile([128, TCOL], F32, tag="ps",
                                        padded_shape=[128, TCOL],
                                        name=f"p{lvl}_{node}_{t}")
                        for x in (0, 1):
                            for q in (0, 1):
                                for sub in (0, 1):
                                    b0 = bs + sub * bh
                                    rhs = curv[:, pn, b0 : b0 + bh, x::2, q::2]
                                    nc.tensor.matmul(
                                        pt[q * 64 : (q + 1) * 64,
                                           sub * 512 : (sub + 1) * 512],
                                        w_t[lvl][:, node * 128 + x * 64 :
                                                 node * 128 + (x + 1) * 64],
                                        rhs,
                                        start=(x == 0), stop=(x == 1),
                                        skip_group_check=True,
                                        tile_position=(0, q * 64),
                                    )
                        evict(
                            nxt2d[:, (node * BG + bs) * Ho * W2o :
                                  (node * BG + bs + bper) * Ho * W2o],
                            pt[:],
                            bias_t[lvl][:, node : node + 1],
                        )
            else:
                # small nodes: multiple nodes per psum tile
                npt = TCOL // cpn        # L3: 2, L4: 8, L5: 32
                wch = None
                for n0 in range(0, n2, npt):
                    pt = ppool.tile([128, npt * cpn], F32, tag="ps",
                                    padded_shape=[128, TCOL],
                                    name=f"p{lvl}_{n0}")
                    for ln in range(npt):
                        node = n0 + ln
                        pn = parent(node)
                        if lvl in (3, 4, 5):
                            # w3/w4/w5 stream just-in-time in 16-node chunks
                            # on the fast (scalar) ring
                            if node % 16 == 0:
                                wch = wpool.tile([128, 2048], BF16, tag="wch",
                                                 name=f"w{lvl}c_{node}")
                                nc.scalar.dma_start(
                                    wch[:],
                                    p[f"w{lvl}"][:, node * 128 : (node + 16) * 128],
                                )
                            wof = (node % 16) * 128
                            wsl = wch
                        else:
                            wof = node * 128
                            wsl = w_t[lvl]
                        for x in (0, 1):
                            for q in (0, 1):
                                rhs = curv[:, pn, :, x::2, q::2]
                                nc.tensor.matmul(
                                    pt[q * 64 : (q + 1) * 64,
                                       ln * cpn : (ln + 1) * cpn],
                                    wsl[:, wof + x * 64 : wof + (x + 1) * 64],
                                    rhs,
                                    start=(x == 0), stop=(x == 1),
                                    skip_group_check=True,
                                    tile_position=(0, q * 64),
                                )
                    if zero_bias:
                        evict(nxt2d[:, n0 * cpn : (n0 + npt) * cpn],
                              pt[:, : npt * cpn])
                    else:
                        for ln in range(npt):
                            node = n0 + ln
                            evict(
                                nxt2d[:, node * cpn : (node + 1) * cpn],
                                pt[:, ln * cpn : (ln + 1) * cpn],
                                bias_t[lvl][:, node : node + 1],
                            )
            cur, cur_nodes = nxt, n2

        # ---------------- level 6 (node pairs, 1x1 out) ----------------
        # feats F2 [128=(sib,c), (pair, b)]
        cur5v = cur[:].rearrange("p (n b h w) -> p n b h w", n=64, b=BG, h=2)
        F2 = fpool.tile([128, 32 * BG], BF16, tag="feats", name="feats")
        F2v = F2[:].rearrange("p (r b) -> p r b", r=32)
        for p0 in range(0, 32, 16):
            pt6 = ppool.tile([128, 16 * BG], F32, tag="ps",
                             padded_shape=[128, TCOL], name=f"p6_{p0}")
            for pr in range(p0, p0 + 16):
                lp = pr - p0
                for x in (0, 1):
                    for half in (0, 1):
                        node = 2 * pr + half
                        rhs = cur5v[:, node, :, x, 0]
                        nc.tensor.matmul(
                            pt6[half * 64 : (half + 1) * 64,
                                lp * BG : (lp + 1) * BG],
                            w_t[6][:, node * 128 + x * 64 :
                                   node * 128 + (x + 1) * 64],
                            rhs,
                            start=(x == 0), stop=(x == 1),
                            skip_group_check=True,
                            tile_position=(0, half * 64),
                        )
            if zero_bias:
                evict(F2[:][:, p0 * BG : (p0 + 16) * BG], pt6[:])
            else:
                for pr in range(p0, p0 + 16):
                    lp = pr - p0
                    evict(F2v[:, pr, :], pt6[:, lp * BG : (lp + 1) * BG],
                          bias_t[6][:, pr : pr + 1])

        # ---------------- dense (row-tiled 2x, parity-major out) ----------
        t2s = fpool.tile([128, NK * NK * BG], BF16, tag="t2s", name="t2s")
        for half in range(2):
            ptd = [
                ppool.tile([128, 256], F32, tag="ps",
                           padded_shape=[128, TCOL], name=f"pd_{s}_{half}")
                for s in range(2)
            ]
            for k in range(half * 16, (half + 1) * 16):
                for s in range(2):
                    rhs = F2v[s * 64 : (s + 1) * 64, k, :]
                    nc.tensor.matmul(
                        ptd[s][:, (k % 16) * BG : (k % 16 + 1) * BG],
                        wd_t[s * 64 : (s + 1) * 64, k * 128 : (k + 1) * 128],
                        rhs,
                        start=True, stop=True,
                        tile_position=(s * 64, 0),
                    )
            for s in range(2):
                dst = t2s[:, s * 512 + half * 256 : s * 512 + (half + 1) * 256]
                if s == 0:
                    nc.scalar.copy(dst, ptd[s][:])
                else:
                    nc.vector.tensor_copy(dst, ptd[s][:])
                nc.scalar.dma_start(
                    t2[:, s * 512 + half * 256 : s * 512 + (half + 1) * 256], dst
                )
    nc.compile()
    return nc


# ----------------------------------------------------------------------------
# entry point
# ----------------------------------------------------------------------------

def kernel(**inputs):
    inputs = {k: np.asarray(v) for k, v in inputs.items()}
    zb = not np.any(inputs["in_bias"]) and all(
        not np.any(inputs[f"b{l}"]) for l in range(1, NLVL + 1)
    )
    wblobs = _prep_weights(inputs)
    nc = _build_kernel(zero_bias=zb)
    in_maps = []
    for c in range(NCORES):
        m = dict(wblobs)
        m["a0"] = _prep_input(inputs["in_data"][c * BC : (c + 1) * BC])
        in_maps.append(m)
    res = run_bass_kernel_spmd(nc, in_maps, list(range(NCORES)))
    outs = [_decode_output(res.results[c]["t2"]) for c in range(NCORES)]
    return np.concatenate(outs, axis=0).astype(np.float32)


if __name__ == "__main__":
    import reference as ref

    inputs = {k: np.asarray(v) for k, v in ref.setup_inputs().items()}
    expected = np.asarray(ref.reference(**inputs))
    actual = kernel(**inputs)
    err = np.abs(actual - expected).max()
    rel = err / np.abs(expected).max()
    print("absmax:", err, "rel:", rel)


# revision 59
# speedup vs baseline: 1.0301x; 1.0067x over previous
"""Trainium2 Bass kernel for nn_ButterflyLayer2D (butterfly 2D CNN).

Strategy: pure data parallel over 8 NeuronCores (16 batch each).

Layouts (per core):
  - activations in SBUF as [128 = (w%2)*64 + c, (node, b, h, w//2)]; every
    2x2-stride-2 per-node conv is 4 bf16 matmuls with K=128=(y,ci):
    q = output w-parity goes to PE column-tile (0, q*64) so the two q
    streams run CONCURRENTLY on the two column halves of the PE array,
    x = input h-parity accumulates in PSUM.  PSUM rows are (q, c_out) ==
    exactly the next level's partition layout, so each eviction is ONE
    full-width relu(+bias) op with contiguous psum read and contiguous
    sbuf write (alternating ScalarE/VectorE).
  - the input 4x4-patch conv packs TWO horizontally adjacent patches per
    rhs column (K=32, block-diagonal lhsT, M=128=(w%2, c)) and row-tiles
    4 ways over (b%4); it is interleaved with level 1 per batch-pair so
    the eviction engines never idle.
  - level 6 (spatial 1x1) computes node pairs with M=(coA,coB);
    features land as [128=(sib,c), (pair, b)].
  - the final dense is row-tiled 2x (even nodes PE rows 0-63, odd 64-127)
    into parity-major psum tiles; output written parity-major and
    decoded on the host.
When all biases are zero (checked on host) psum tiles covering several
nodes are evicted in one op; otherwise per-node evicts apply the bias.
Weights are prefetched whole-level across both HWDGE rings; w3 streams
just-in-time in 8-node chunks.
"""

import numpy as np
from contextlib import ExitStack

import concourse.bass as bass
import concourse.tile as tile
from concourse import bacc, mybir
from concourse.bass_utils import run_bass_kernel_spmd

F32 = mybir.dt.float32
BF16 = mybir.dt.bfloat16
AF = mybir.ActivationFunctionType
ALU = mybir.AluOpType

B, IN, NLVL, KLVL, C = 128, 256, 6, 3, 64
NK, OU, OV = 8, 8, 8
NCORES = 8
BC = B // NCORES          # 16 per-core batch
PH = 1
BG = BC
TCOL = 1024               # psum tile columns (2 banks)
PBUFS = 4
LVL_NODES = [4, 16, 64, 64, 64, 64]
LVL_HIN = [64, 32, 16, 8, 4, 2]


# ----------------------------------------------------------------------------
# host-side pre-arrangement
# ----------------------------------------------------------------------------

def _prep_weights(inputs):
    """Weights/biases blobs shared by all cores."""
    import ml_dtypes
    out = {}
    # input filter: per (g=b%4, q=w%2) a [128, 64] lhsT that is zero except
    # rows g*32+q*16 .. +16 = fin[pix, c] — K=128 mms in the same
    # 128x64 column-tiled mode as the levels (no PE mode switches)
    fin = inputs["in_filter"][:, :, 0, :].reshape(16, C).astype(np.float32)
    finr = np.zeros((128, 8, C), np.float32)
    for g in range(4):
        for q in range(2):
            finr[g * 32 + q * 16 : g * 32 + (q + 1) * 16, g * 2 + q] = fin
    out["fin"] = finr.reshape(128, 8 * C).astype(ml_dtypes.bfloat16)
    # misc blob: [bin | b1 | b2 | b3 | b4 | b5 | b6]
    misc = [np.concatenate([inputs["in_bias"], inputs["in_bias"]]).reshape(128, 1)]
    for lvl in range(1, NLVL + 1):
        f = inputs[f"f{lvl}"].astype(np.float32)  # [n,n,2,2,C,C] (x,y,ci,co)
        n = f.shape[0]
        # per node lhsT [(y,ci)=128, (x,co)=128]
        w = f.transpose(0, 1, 3, 4, 2, 5).reshape(n * n, 2 * C, 2 * C)
        out[f"w{lvl}"] = np.ascontiguousarray(w.transpose(1, 0, 2)).reshape(
            128, n * n * 128
        ).astype(ml_dtypes.bfloat16)
        b = inputs[f"b{lvl}"].astype(np.float32).reshape(n * n, C)
        if lvl < NLVL:
            misc.append(np.concatenate([b, b], axis=1).T)  # [(q,c)=128, nodes]
        else:
            misc.append(b.reshape(n * n // 2, 2 * C).T)    # [(cA,cB), pairs]
    out["misc"] = np.ascontiguousarray(
        np.concatenate(misc, axis=1)
    ).astype(np.float32)
    # dense: [128, pairs*128]: rows 0-63 even-node [c,(r,ouov)], 64-127 odd
    wd = inputs["Wd"].astype(np.float32).reshape(NK * NK, 2, C, OU * OV)
    wd = wd.transpose(2, 0, 1, 3).reshape(C, NK * NK, 2 * OU * OV)
    wd2 = np.zeros((128, NK * NK // 2 * 128), np.float32)
    for k in range(NK * NK // 2):
        wd2[0:64, k * 128 : (k + 1) * 128] = wd[:, 2 * k]
        wd2[64:128, k * 128 : (k + 1) * 128] = wd[:, 2 * k + 1]
    out["wd"] = wd2.astype(ml_dtypes.bfloat16)
    return out


def _prep_input(in_data_core):
    """Per-core input blob [128 = (b%4)*32 + (j%8//4)*16 + (i%4)*4 + (j%4),
    (b//4, x=i//4, y2=j//8)]."""
    import ml_dtypes
    ind = in_data_core[:, :, :, 0]  # [16, 256, 256]
    a = ind.reshape(4, 4, 64, 4, 32, 2, 4)  # [half, g, x, p, y2, yp, q]
    a = a.transpose(1, 5, 3, 6, 0, 2, 4)    # [g, yp, p, q, half, x, y2]
    return np.ascontiguousarray(a).reshape(128, 4 * 64 * 32).astype(
        ml_dtypes.bfloat16
    )


def _decode_output(t2_core):
    """t2 [128=(r,ou,ov), (s, k, b)] with node = 2k+s -> [16, 64, 64, 2]."""
    t = t2_core.reshape(2, OU, OV, 2, 8, 4, BG)  # r,ou,ov,s,u,v2,b
    t = t.transpose(6, 4, 1, 5, 3, 2, 0)         # b,u,ou,v2,s,ov,r
    return np.ascontiguousarray(t).reshape(BC, NK * OU, NK * OV, 2)


# ----------------------------------------------------------------------------
# device kernel
# ----------------------------------------------------------------------------

def _build_kernel(zero_bias=True):
    nc = bacc.Bacc(None, target_bir_lowering=False)
    p = {}
    p["a0"] = nc.declare_dram_parameter("a0", [128, 4 * 64 * 32], BF16, isOutput=False)
    p["fin"] = nc.declare_dram_parameter("fin", [128, 512], BF16, isOutput=False)
    # misc f32 blob: [bin(1) | b1(4) | b2(16) | b3..b5(64 each) | b6(32)]
    p["misc"] = nc.declare_dram_parameter("misc", [128, 245], F32, isOutput=False)
    for lvl in range(1, NLVL + 1):
        n2 = LVL_NODES[lvl - 1]
        p[f"w{lvl}"] = nc.declare_dram_parameter(f"w{lvl}", [128, n2 * 128], BF16, isOutput=False)
    p["wd"] = nc.declare_dram_parameter("wd", [128, NK * NK // 2 * 128], BF16, isOutput=False)
    t2 = nc.declare_dram_parameter("t2", [128, NK * NK * BG], BF16, isOutput=True)

    evict_ctr = [0]

    def evict(out_ap, psum_ap, bias_ap=None):
        """relu(psum [+ bias]) -> sbuf, alternating engines."""
        evict_ctr[0] += 1
        if evict_ctr[0] % 13 % 2 == 0:
            if bias_ap is None:
                nc.scalar.activation(out_ap, psum_ap, AF.Relu)
            else:
                nc.scalar.activation(out_ap, psum_ap, AF.Relu, bias=bias_ap)
        elif bias_ap is None:
            nc.vector.tensor_scalar_max(out_ap, psum_ap, 0.0)
        else:
            nc.vector.tensor_scalar(out_ap, psum_ap, bias_ap, 0.0,
                                    op0=ALU.add, op1=ALU.max)

    with tile.TileContext(nc) as tc, ExitStack() as ctx:
        const = ctx.enter_context(tc.tile_pool(name="const", bufs=1))
        wbig = ctx.enter_context(tc.tile_pool(name="wbig", bufs=1))
        wpool = ctx.enter_context(tc.tile_pool(name="wts", bufs=4))
        apool = ctx.enter_context(tc.tile_pool(name="acts", bufs=1))
        inpool = ctx.enter_context(tc.tile_pool(name="inp", bufs=1))
        fpool = ctx.enter_context(tc.tile_pool(name="feat", bufs=1))
        ppool = ctx.enter_context(tc.tile_pool(name="ps", bufs=PBUFS, space="PSUM"))

        # ---------------- DMA prologue ----------------
        # scalar ring (fast q10): a0 halves 0,1 then big weights w5,w6,wd,w4
        # sync ring: consts, w1, w2; later a0 halves 2,3 + w3 chunks in-loop
        a0s = inpool.tile([128, 3 * 64 * 32], BF16, tag="a0s", name="a0s")
        # scalar ring (fast) carries the critical stream in dependency
        # order: input half 0, w1, input half 1, w2; big late weights are
        # issued from inside the pipeline loop so they queue behind the
        # staging transfers, not ahead of them.
        nc.scalar.dma_start(a0s[:, 0:1024], p["a0"][:, 0:1024])
        fin_t = const.tile([128, 512], BF16)
        nc.sync.dma_start(fin_t[:], p["fin"][:])
        misc_t = const.tile([128, 245], F32, tag="misc", name="misc")
        nc.sync.dma_start(misc_t[:], p["misc"][:])
        bin_t = misc_t[:, 0:1]
        bias_t = {}
        boff = 1
        for lvl in range(1, NLVL + 1):
            bcols = LVL_NODES[lvl - 1] if lvl < NLVL else LVL_NODES[lvl - 1] // 2
            bias_t[lvl] = misc_t[:, boff : boff + bcols]
            boff += bcols
        nc.scalar.dma_start(a0s[:, 1024:2048], p["a0"][:, 1024:2048])
        w_t = {}
        w_t[1] = wbig.tile([128, 4 * 128], BF16, tag="w1", name="w1")
        nc.scalar.dma_start(w_t[1][:], p["w1"][:])
        nc.scalar.dma_start(a0s[:, 2048:4096], p["a0"][:, 2048:4096])
        w_t[2] = wbig.tile([128, 16 * 128], BF16, tag="w2", name="w2")
        nc.scalar.dma_start(w_t[2][:], p["w2"][:])
        nc.scalar.dma_start(a0s[:, 4096:6144], p["a0"][:, 4096:6144])
        w_t[6] = wbig.tile([128, 64 * 128], BF16, tag="w6", name="w6")
        wd_t = wbig.tile([128, NK * NK // 2 * 128], BF16, tag="wd", name="wd")

        a0v = a0s[:].rearrange("p (h x y) -> p h x y", h=3, x=64)

        # ------- input conv + level 1 + level 2, software-pipelined -------
        # X is a 4-slice ring over b%4 (L1 consumes each b right away):
        # [128=(w%2,c), (b%4, h=64, w2=32)]; L1 out: [128, (n=4, b, 32, 16)]
        # L2 out: [128, (n=16, b, 16, 8)]
        X = apool.tile([128, 4 * 64 * 32], BF16, tag="sx", name="x0")
        X2d = X[:]
        Xv = X[:].rearrange("p (b h w) -> p b h w", b=4, h=64)
        a1 = apool.tile([128, 4 * BG * 32 * 16], BF16, tag="s1", name="a1")
        a12d = a1[:]
        a1v = a1[:].rearrange("p (n b h w) -> p n b h w", n=4, b=BG, h=32)
        a2 = apool.tile([128, 16 * BG * 16 * 8], BF16, tag="s2", name="a2")
        a22d = a2[:]

        def input_pair(pr):
            for bl in (2 * pr, 2 * pr + 1):
                g, hh = bl % 4, (bl // 4) % 3
                for xh in range(2):
                    pt = ppool.tile([128, TCOL], F32, tag="ps",
                                    padded_shape=[128, TCOL],
                                    name=f"pin_{bl}_{xh}")
                    for sub in range(2):
                        rhs = a0v[:, hh,
                                  xh * 32 + sub * 16 : xh * 32 + (sub + 1) * 16, :]
                        for q in (0, 1):
                            nc.tensor.matmul(
                                pt[q * 64 : (q + 1) * 64,
                                   sub * 512 : (sub + 1) * 512],
                                fin_t[:, (g * 2 + q) * 64 : (g * 2 + q + 1) * 64],
                                rhs,
                                start=True, stop=True,
                                tile_position=(0, q * 64),
                            )
                    evict(
                        X2d[:, ((bl % 4) * 64 + xh * 32) * 32 :
                            ((bl % 4) * 64 + (xh + 1) * 32) * 32],
                        pt[:], bin_t,
                    )

        def l1_tiles(pr):
            # 4 nodes for batch pair pr (parent is the whole X)
            bs = 2 * pr
            for node in range(4):
                pt = ppool.tile([128, TCOL], F32, tag="ps",
                                padded_shape=[128, TCOL],
                                name=f"p1_{node}_{pr}")
                for x in (0, 1):
                    for q in (0, 1):
                        for sub in (0, 1):
                            rhs = Xv[:, (bs + sub) % 4, x::2, q::2]
                            nc.tensor.matmul(
                                pt[q * 64 : (q + 1) * 64,
                                   sub * 512 : (sub + 1) * 512],
                                w_t[1][:, node * 128 + x * 64 :
                                       node * 128 + (x + 1) * 64],
                                rhs,
                                start=(x == 0), stop=(x == 1),
                                skip_group_check=True,
                                tile_position=(0, q * 64),
                            )
                evict(
                    a12d[:, (node * BG + bs) * 512 : (node * BG + bs + 2) * 512],
                    pt[:],
                    bias_t[1][:, node : node + 1],
                )

        def l2_tile(node, bs):
            # one [128,1024] tile: 8 batches of one node; parent in a1
            pn = (node // 4 // 2) * 2 + (node % 4) // 2
            pt = ppool.tile([128, TCOL], F32, tag="ps",
                            padded_shape=[128, TCOL],
                            name=f"p2_{node}_{bs}")
            for x in (0, 1):
                for q in (0, 1):
                    for sub in (0, 1):
                        b0 = bs + sub * 4
                        rhs = a1v[:, pn, b0 : b0 + 4, x::2, q::2]
                        nc.tensor.matmul(
                            pt[q * 64 : (q + 1) * 64,
                               sub * 512 : (sub + 1) * 512],
                            w_t[2][:, node * 128 + x * 64 :
                                   node * 128 + (x + 1) * 64],
                            rhs,
                            start=(x == 0), stop=(x == 1),
                            skip_group_check=True,
                            tile_position=(0, q * 64),
                        )
            evict(
                a22d[:, (node * BG + bs) * 128 : (node * BG + bs + 8) * 128],
                pt[:],
                bias_t[2][:, node : node + 1],
            )

        for pr in range(8):
            # stage the last a0 half over slot 0 once batch pair 0-1 is done
            if pr == 4:
                nc.scalar.dma_start(a0s[:, 0:2048], p["a0"][:, 6144:8192])
            elif pr == 5:
                # big late weights queue on the fast ring behind the staging
                nc.scalar.dma_start(w_t[6][:], p["w6"][:])
                nc.scalar.dma_start(wd_t[:], p["wd"][:])

            input_pair(pr)
            l1_tiles(pr)
            if pr >= 4:
                # L1 b0-7 is complete; start L2 (bs=0) tiles
                for node in range(4 * (pr - 4), 4 * (pr - 3)):
                    l2_tile(node, 0)
        for node in range(16):
            l2_tile(node, 8)

        # ---------------- levels 3..5 (q-scheme) ----------------
        cur, cur_nodes = a2, 16
        tags = ["s1", "s2", "s1"]
        for lvl in range(3, 6):
            n2 = LVL_NODES[lvl - 1]
            grid = int(np.sqrt(n2))
            Hin = LVL_HIN[lvl - 1]
            Ho, W2o = Hin // 2, Hin // 4
            cpn = BG * Ho * W2o          # psum cols per node
            pgrid = int(np.sqrt(cur_nodes))
            nxt = apool.tile([128, n2 * cpn], BF16, tag=tags[lvl - 3],
                             name=f"a{lvl}")
            nxt2d = nxt[:]
            curv = cur[:].rearrange("p (n b h w) -> p n b h w",
                                    n=cur_nodes, b=BG, h=Hin)

            def parent(node):
                if lvl > KLVL:
                    return node
                u, v = node // grid, node % grid
                return (u // 2) * pgrid + (v // 2)

            if cpn >= TCOL:
                # large nodes (L2: 2 tiles/node split along b)
                tpn = cpn // TCOL
                bper = TCOL // (Ho * W2o)
                bh = bper // 2
                for node in range(n2):
                    pn = parent(node)
                    for t in range(tpn):
                        bs = t * bper
                        pt = ppool.tile([128, TCOL], F32, tag="ps",
                                        padded_shape=[128, TCOL],
                                        name=f"p{lvl}_{node}_{t}")
                        for x in (0, 1):
                            for q in (0, 1):
                                for sub in (0, 1):
                                    b0 = bs + sub * bh
                                    rhs = curv[:, pn, b0 : b0 + bh, x::2, q::2]
                                    nc.tensor.matmul(
                                        pt[q * 64 : (q + 1) * 64,
                                           sub * 512 : (sub + 1) * 512],
                                        w_t[lvl][:, node * 128 + x * 64 :
                                                 node * 128 + (x + 1) * 64],
                                        rhs,
                                        start=(x == 0), stop=(x == 1),
                                        skip_group_check=True,
                                        tile_position=(0, q * 64),
                                    )
                        evict(
                            nxt2d[:, (node * BG + bs) * Ho * W2o :
                                  (node * BG + bs + bper) * Ho * W2o],
                            pt[:],
                            bias_t[lvl][:, node : node + 1],
                        )
            else:
                # small nodes: multiple nodes per psum tile
                npt = TCOL // cpn        # L3: 2, L4: 8, L5: 32
                wch = None
                for n0 in range(0, n2, npt):
                    pt = ppool.tile([128, npt * cpn], F32, tag="ps",
                                    padded_shape=[128, TCOL],
                                    name=f"p{lvl}_{n0}")
                    for ln in range(npt):
                        node = n0 + ln
                        pn = parent(node)
                        if lvl in (3, 4, 5):
                            # w3/w4/w5 stream just-in-time in 16-node chunks
                            # on the fast (scalar) ring
                            if node % 16 == 0:
                                wch = wpool.tile([128, 2048], BF16, tag="wch",
                                                 name=f"w{lvl}c_{node}")
                                nc.scalar.dma_start(
                                    wch[:],
                                    p[f"w{lvl}"][:, node * 128 : (node + 16) * 128],
                                )
                            wof = (node % 16) * 128
                            wsl = wch
                        else:
                            wof = node * 128
                            wsl = w_t[lvl]
                        for x in (0, 1):
                            for q in (0, 1):
                                rhs = curv[:, pn, :, x::2, q::2]
                                nc.tensor.matmul(
                                    pt[q * 64 : (q + 1) * 64,
                                       ln * cpn : (ln + 1) * cpn],
                                    wsl[:, wof + x * 64 : wof + (x + 1) * 64],
                                    rhs,
                                    start=(x == 0), stop=(x == 1),
                                    skip_group_check=True,
                                    tile_position=(0, q * 64),
                                )
                    if zero_bias:
                        evict(nxt2d[:, n0 * cpn : (n0 + npt) * cpn],
                              pt[:, : npt * cpn])
                    else:
                        for ln in range(npt):
                            node = n0 + ln
                            evict(
                                nxt2d[:, node * cpn : (node + 1) * cpn],
                                pt[:, ln * cpn : (ln + 1) * cpn],
                                bias_t[lvl][:, node : node + 1],
                            )
            cur, cur_nodes = nxt, n2

        # ---------------- level 6 (node pairs, 1x1 out) ----------------
        # feats F2 [128=(sib,c), (pair, b)]
        cur5v = cur[:].rearrange("p (n b h w) -> p n b h w", n=64, b=BG, h=2)
        F2 = fpool.tile([128, 32 * BG], BF16, tag="feats", name="feats")
        F2v = F2[:].rearrange("p (r b) -> p r b", r=32)
        for p0 in range(0, 32, 16):
            pt6 = ppool.tile([128, 16 * BG], F32, tag="ps",
                             padded_shape=[128, TCOL], name=f"p6_{p0}")
            for pr in range(p0, p0 + 16):
                lp = pr - p0
                for x in (0, 1):
                    for half in (0, 1):
                        node = 2 * pr + half
                        rhs = cur5v[:, node, :, x, 0]
                        nc.tensor.matmul(
                            pt6[half * 64 : (half + 1) * 64,
                                lp * BG : (lp + 1) * BG],
                            w_t[6][:, node * 128 + x * 64 :
                                   node * 128 + (x + 1) * 64],
                            rhs,
                            start=(x == 0), stop=(x == 1),
                            skip_group_check=True,
                            tile_position=(0, half * 64),
                        )
            if zero_bias:
                evict(F2[:][:, p0 * BG : (p0 + 16) * BG], pt6[:])
            else:
                for pr in range(p0, p0 + 16):
                    lp = pr - p0
                    evict(F2v[:, pr, :], pt6[:, lp * BG : (lp + 1) * BG],
                          bias_t[6][:, pr : pr + 1])

        # ---------------- dense (row-tiled 2x, parity-major out) ----------
        t2s = fpool.tile([128, NK * NK * BG], BF16, tag="t2s", name="t2s")
        for half in range(2):
            ptd = [
                ppool.tile([128, 256], F32, tag="ps",
                           padded_shape=[128, TCOL], name=f"pd_{s}_{half}")
                for s in range(2)
            ]
            for k in range(half * 16, (half + 1) * 16):
                for s in range(2):
                    rhs = F2v[s * 64 : (s + 1) * 64, k, :]
                    nc.tensor.matmul(
                        ptd[s][:, (k % 16) * BG : (k % 16 + 1) * BG],
                        wd_t[s * 64 : (s + 1) * 64, k * 128 : (k + 1) * 128],
                        rhs,
                        start=True, stop=True,
                        tile_position=(s * 64, 0),
                    )
            for s in range(2):
                dst = t2s[:, s * 512 + half * 256 : s * 512 + (half + 1) * 256]
                if s == 0:
                    nc.scalar.copy(dst, ptd[s][:])
                else:
                    nc.vector.tensor_copy(dst, ptd[s][:])
                ring = nc.scalar if s == 0 else nc.sync
                ring.dma_start(
                    t2[:, s * 512 + half * 256 : s * 512 + (half + 1) * 256], dst
                )
    nc.compile()
    return nc


# ----------------------------------------------------------------------------
# entry point
# ----------------------------------------------------------------------------

def kernel(**inputs):
    inputs = {k: np.asarray(v) for k, v in inputs.items()}
    zb = not np.any(inputs["in_bias"]) and all(
        not np.any(inputs[f"b{l}"]) for l in range(1, NLVL + 1)
    )
    wblobs = _prep_weights(inputs)
    nc = _build_kernel(zero_bias=zb)
    in_maps = []
    for c in range(NCORES):
        m = dict(wblobs)
        m["a0"] = _prep_input(inputs["in_data"][c * BC : (c + 1) * BC])
        in_maps.append(m)
    res = run_bass_kernel_spmd(nc, in_maps, list(range(NCORES)))
    outs = [_decode_output(res.results[c]["t2"]) for c in range(NCORES)]
    return np.concatenate(outs, axis=0).astype(np.float32)


if __name__ == "__main__":
    import reference as ref

    inputs = {k: np.asarray(v) for k, v in ref.setup_inputs().items()}
    expected = np.asarray(ref.reference(**inputs))
    actual = kernel(**inputs)
    err = np.abs(actual - expected).max()
    rel = err / np.abs(expected).max()
    print("absmax:", err, "rel:", rel)
